# revision 30
# baseline (speedup 1.0000x reference)
"""Local windowed attention (window=128, look back/forward 1) on 8 trn2 cores.

v2 design. Data-parallel over 32 (b*h) head-slices, 4 per core, processed
as 2 slice-PAIRS per core.  For each pair, q/k live d-major in one SBUF
tile of 128 partitions: slice 2j on partitions 0-63, slice 2j+1 on 64-127.
Score matmuls for the two slices run CONCURRENTLY as row-tiles of the PE
array (tile_position (0,0) / (64,0), K=64 each) -> 2x MM1 throughput with
no input duplication.

Scores land as S^T (keys on partitions, queries free).  exp is split
across three engines to break the ACT throughput wall (~53us if ACT-only):
  - ACT: exact table exp,   (768+222)cyc @1.2GHz per chunk-unit
  - DVE/Pool: dual-phase Schraudolph exp2: two int32 affine images of the
    scores bitcast to f32 and multiplied; phase-offset biases cancel the
    linear-interp ripple to ~±1.5%, bias-corrected to be mixable with
    exact-exp chunks (validated: adds ~4e-3 final rel err).
PV keeps queries-on-partitions output via E^T-stationary matmuls with a
ones-column in v producing the softmax denominator in psum column 64.
Finalize (reciprocal + broadcast multiply) is batched 4 windows per DVE
instruction pair. Boundary chunks stream a clamped 3-window q range so
every psum byte is written (uniform shapes, CoreSim-clean).

A numpy fallback handles non-all-True masks (graded fill is all-True).
"""

import os
import sys

import numpy as np

for _p in ("/root/.axon_site", "/root/.axon_site/_ro/trn_rl_repo",
           "/root/.axon_site/_ro/pypackages", "/opt/trn_rl_repo", "/opt/pypackages"):
    if os.path.isdir(_p) and _p not in sys.path:
        sys.path.append(_p)

from concourse import bacc
import concourse.mybir as mybir
import concourse.tile as tile
from concourse.bass_utils import run_bass_kernel_spmd

B, N, DM = 4, 4096, 512
H, D = 8, 64
WIN = 128
NW = N // WIN            # 32 windows
NCORES = 8
HPC = B * H // NCORES    # head-slices per core = 4
NPAIR = HPC // 2         # slice-pairs per core = 2
SCALE = DM ** -0.5

F32 = mybir.dt.float32
BF16 = mybir.dt.bfloat16
I16 = mybir.dt.int16

# dual-phase Schraudolph constants (validated in numpy: mixed-mode adds
# ~4e-3 max rel err on the final output; see session notes)
LOG2E = 1.4426950408889634
S_BIAS = 0.111           # c1 + c2: compensates the mean interp overshoot
C1 = (S_BIAS - 0.5) / 2.0
C2 = C1 + 0.5
# bf16-domain variant: int16 images of the bf16 bit lattice (exponent at
# bit 7).  2-byte in/out keeps the DVE ops in 2x perf mode.
EXP_A = float(np.float32(SCALE * LOG2E * (1 << 6)))
EXP_B1 = float(np.float32((127.0 - C1) * (1 << 7)))
EXP_B2 = float(np.float32((127.0 - C2) * (1 << 7)))

# exp engine per chunk (pattern repeated per pair): 'A' ACT exact,
# 'P' Pool dual-phase (via a DVE psum->sbuf staging copy: GpSimd has no
# PSUM port), 'D' DVE dual-phase (reads psum directly).
TRICK = {1: 'P', 4: 'P', 7: 'P', 10: 'P', 13: 'P', 16: 'P', 19: 'P',
         22: 'P', 25: 'P'}

# PV of unit u is emitted (and scheduled) PV_SKEW units behind its MM1 so
# the PE never head-of-line blocks on the slowest exp engine's latency.
# Trick units split: i1 on DVE (slot u), i2 on Pool, product on DVE but
# emitted TT_SKEW units later so the DVE FIFO never waits on Pool.
PV_SKEW = 12
TT_SKEW = 6
GW = 3                   # windows per pv psum bank (x2 heads = 390 fp32)
NG = (NW + GW - 1) // GW


def _unit_engine(c):
    return TRICK.get(c, 'A')


def _build_program(repeat=1, unroll=False):
    nc = bacc.Bacc(trn_type="TRN2")
    qt = nc.dram_tensor("qt", (NPAIR, 128, N), BF16, kind="ExternalInput")
    kt = nc.dram_tensor("kt", (NPAIR, 128, N), BF16, kind="ExternalInput")
    vx = nc.dram_tensor("vx", (NPAIR, WIN, 2, NW, D + 1), BF16,
                        kind="ExternalInput")
    # 65-wide output rows: finalize multiplies the full pv bank row
    # (64 outputs + denominator column) so its APs flatten to <=3D for
    # walrus; the host slices off column 64.
    out = nc.dram_tensor("out", (NPAIR, WIN, 2, NW, D + 1), BF16,
                         kind="ExternalOutput")

    mult = mybir.AluOpType.mult
    add = mybir.AluOpType.add

    with tile.TileContext(nc) as tc:
        with (
            tc.tile_pool(name="inp", bufs=2) as inp,
            tc.tile_pool(name="ex", bufs=PV_SKEW + 2) as exp_pool,
            tc.tile_pool(name="i32", bufs=4) as i32_pool,
            tc.tile_pool(name="fin", bufs=4) as fin,
            tc.tile_pool(name="ob", bufs=2) as obp,
            tc.tile_pool(name="ps_s", bufs=3, space="PSUM") as ps_s,
            tc.tile_pool(name="ps_pv", bufs=2, space="PSUM") as ps_pv,
        ):
            pairs = [None] * (NPAIR + 1)
            state = {}

            def load_pair(j):
                if j >= NPAIR:
                    return
                q_sb = inp.tile([128, N], BF16, tag="q", name=f"q_sb{j}")
                k_sb = inp.tile([128, N], BF16, tag="k", name=f"k_sb{j}")
                v_sb = inp.tile([WIN, 2, NW, D + 1], BF16, tag="v",
                                name=f"v_sb{j}")
                bounds = [0, 1024, 2048, 3072, 4096]
                for sl in range(4):
                    csl = slice(bounds[sl], bounds[sl + 1])
                    nc.sync.dma_start(out=k_sb[:, csl], in_=kt[j, :, csl])
                    nc.sync.dma_start(out=q_sb[:, csl], in_=qt[j, :, csl])
                nc.sync.dma_start(out=v_sb[:, 0], in_=vx[j, :, 0])
                nc.sync.dma_start(out=v_sb[:, 1], in_=vx[j, :, 1])
                pairs[j] = (q_sb, k_sb, v_sb)

            def emit_mm1(j, c):
                q_sb, k_sb, _ = pairs[j]
                sT2 = ps_s.tile([WIN, 2, 512], F32, space="PSUM", tag="sT2",
                                name=f"sT2_{j}_{c}")
                base = min(max(c - 1, 0), NW - 3) * WIN
                for h in (0, 1):
                    nc.tensor.matmul(
                        sT2[:, h, :3 * WIN],
                        lhsT=k_sb[64 * h:64 * (h + 1), c * WIN:(c + 1) * WIN],
                        rhs=q_sb[64 * h:64 * (h + 1), base:base + 3 * WIN],
                        start=True, stop=True,
                    )
                return sT2

            def emit_exp(j, c, sT2):
                # Returns either a finished ex2 (ACT path) or the pending
                # (i1, i2) pair to be combined by emit_exp_tt later.
                eng = _unit_engine(c)
                src = sT2[:, :, :3 * WIN]
                if eng == 'A':
                    ex2 = exp_pool.tile([WIN, 2, 3 * WIN], BF16, tag="ex2",
                                        name=f"ex2_{j}_{c}")
                    nc.scalar.activation(
                        ex2, src, mybir.ActivationFunctionType.Exp,
                        scale=SCALE,
                    )
                    return ('done', ex2)
                # trick unit: i1 = rint(s*A + B1) as int16 (bf16 lattice),
                # straight from psum on DVE (single pass, frees the score
                # bank); i2 = i1 - 64 on Pool (exact half-octave phase
                # shift: rint(x - 64) == rint(x) - 64).
                i1 = i32_pool.tile([WIN, 2, 3 * WIN], I16, tag="i1",
                                   name=f"i1_{j}_{c}")
                i2 = i32_pool.tile([WIN, 2, 3 * WIN], I16, tag="i2",
                                   name=f"i2_{j}_{c}")
                nc.vector.tensor_scalar(i1, src, EXP_A, EXP_B1,
                                        op0=mult, op1=add)
                nc.gpsimd.tensor_scalar_add(i2, i1, -64.0)
                return ('pend', (j, c, i1, i2))

            def emit_exp_tt(pend):
                _, (j, c, i1, i2) = pend
                ex2 = exp_pool.tile([WIN, 2, 3 * WIN], BF16, tag="ex2",
                                    name=f"ex2_{j}_{c}")
                nc.vector.tensor_tensor(ex2, i1.bitcast(BF16),
                                        i2.bitcast(BF16), op=mult)
                return ('done', ex2)

            def finalize_group(j, g):
                st = state[j]
                pv = st['pv'].pop(g)
                gw = pv.shape[2]
                rc = fin.tile([WIN, 2, GW, 1], F32, tag="rc",
                              name=f"rc_{j}_{g}")[:, :, :gw]
                nc.vector.reciprocal(rc, pv[:, :, :, D:D + 1])
                ob = st['ob']
                for h in (0, 1):
                    # per-head so every AP flattens to <=3D: in0/out are
                    # contiguous [gw*65] rows, in1 broadcasts rc over 65
                    nc.vector.scalar_tensor_tensor(
                        out=ob[:, h, g * GW:g * GW + gw, :],
                        in0=pv[:, h],
                        scalar=1.0,
                        in1=rc[:, h].broadcast_to([WIN, gw, D + 1]),
                        op0=mult, op1=mult,
                    )
                if g == NG - 1:
                    nc.sync.dma_start(out=out[j], in_=ob)

            def emit_pv(j, c, ex2):
                _, _, v_sb = pairs[j]
                st = state[j]
                base_w = min(max(c - 1, 0), NW - 3)
                for h in (0, 1):
                    for w in range(max(0, c - 1), min(NW - 1, c + 1) + 1):
                        first = c == max(0, w - 1)
                        last = c == min(NW - 1, w + 1)
                        g = w // GW
                        gw = min(GW, NW - g * GW)
                        if first and w % GW == 0 and h == 0:
                            st['pv'][g] = ps_pv.tile(
                                [WIN, 2, gw, D + 1], F32, space="PSUM",
                                tag="pv", padded_shape=[WIN, 2, GW, D + 1],
                                name=f"pv_{j}_{g}",
                            )
                        blk = w - base_w
                        # start=True clears has_written for the WHOLE psum
                        # bank, so only the bank's first matmul may carry it;
                        # other slots' first writes overwrite via the
                        # pending-zero bytes.  stop likewise only on the
                        # bank's final matmul (sim group bookkeeping).
                        nc.tensor.matmul(
                            st['pv'][g][:, h, w % GW, :],
                            lhsT=ex2[:, h, blk * WIN:(blk + 1) * WIN],
                            rhs=v_sb[:, h, c, :],
                            start=first and w % GW == 0 and h == 0,
                            stop=last and w % GW == gw - 1 and h == 1,
                        )
                        if last and w % GW == gw - 1 and h == 1:
                            finalize_group(j, g)

            # emission skews: exp(u-1) after MM1(u); PV(u-PV_SKEW) last.
            def one_iteration():
                stages = [(j, c) for j in range(NPAIR) for c in range(NW)]
                load_pair(0)
                sT2s = {}
                ex2s = {}
                for u, (j, c) in enumerate(stages):
                    if c == 0:
                        load_pair(j + 1)
                        state[j] = {
                            'pv': {},
                            'ob': obp.tile([WIN, 2, NW, D + 1], BF16,
                                           tag="ob", name=f"ob_{j}"),
                        }
                    sT2s[u] = emit_mm1(j, c)
                    if u >= 1:
                        ju, cu = stages[u - 1]
                        ex2s[u - 1] = emit_exp(ju, cu, sT2s.pop(u - 1))
                    x = u - 1 - TT_SKEW
                    if x >= 0 and ex2s[x][0] == 'pend':
                        ex2s[x] = emit_exp_tt(ex2s[x])
                    if u >= PV_SKEW:
                        x = u - PV_SKEW
                        ju, cu = stages[x]
                        emit_pv(ju, cu, ex2s.pop(x)[1])
                nu = len(stages)
                ju, cu = stages[nu - 1]
                ex2s[nu - 1] = emit_exp(ju, cu, sT2s.pop(nu - 1))
                for x in sorted(ex2s):
                    if ex2s[x][0] == 'pend':
                        ex2s[x] = emit_exp_tt(ex2s[x])
                for x in sorted(ex2s):
                    ju, cu = stages[x]
                    emit_pv(ju, cu, ex2s.pop(x)[1])

            if unroll:
                for _ in range(repeat):
                    one_iteration()
            elif repeat > 1:
                with tc.For_i(0, repeat, 1):
                    one_iteration()
            else:
                one_iteration()
    nc.finalize()
    return nc


_NC = None


def _get_nc():
    global _NC
    if _NC is None:
        _NC = _build_program()
    return _NC


def _shard_inputs(q, k, v):
    q = np.ascontiguousarray(q, np.float32)
    k = np.ascontiguousarray(k, np.float32)
    v = np.ascontiguousarray(v, np.float32)

    import ml_dtypes

    def split_t(x):  # (B,N,DM) -> (B*H, D, N) d-major, bf16
        x = x.reshape(B, N, H, D).transpose(0, 2, 3, 1)
        x = np.ascontiguousarray(x).reshape(B * H, D, N)
        return x.astype(ml_dtypes.bfloat16)

    # pairs: (NCORES, NPAIR, 128, N): slice 2j on partitions 0-63, 2j+1 on 64-127
    qt = split_t(q).reshape(NCORES, NPAIR, 128, N)
    kt = split_t(k).reshape(NCORES, NPAIR, 128, N)

    vv = v.reshape(B, N, H, D).transpose(0, 2, 1, 3).reshape(B * H, N, D)
    vx = np.concatenate([vv, np.ones((B * H, N, 1), np.float32)], axis=2)
    # (B*H, NW, WIN, D+1) -> per-slice [WIN part, NW, D+1]
    vx = vx.reshape(B * H, NW, WIN, D + 1).transpose(0, 2, 1, 3)
    # group into pairs: (NCORES, NPAIR, 2, WIN, NW, D+1) -> (c, j, WIN, 2, NW, D+1)
    vx = vx.reshape(NCORES, NPAIR, 2, WIN, NW, D + 1).transpose(0, 1, 3, 2, 4, 5)
    vx = np.ascontiguousarray(vx).astype(ml_dtypes.bfloat16)

    return [
        {"qt": qt[c], "kt": kt[c], "vx": vx[c]}
        for c in range(NCORES)
    ]


def _unshard_output(per_core):
    # per-core out: (NPAIR, WIN, 2, NW, D+1); slice idx = core*4 + j*2 + h
    o = np.stack(per_core).astype(np.float32)[..., :D]
    o = o.transpose(0, 1, 3, 2, 4, 5)          # (c, j, h, WIN, NW, D)
    o = o.reshape(B, H, WIN, NW, D).transpose(0, 3, 2, 1, 4)  # b nw win h d
    return np.ascontiguousarray(o).reshape(B, N, DM)


def _numpy_fallback(q, k, v, mask):
    # Faithful replication of the reference for non-all-true masks.
    w = N // WIN
    scale = SCALE

    def split(x):
        x = x.reshape(B, w, WIN, H, D)
        return x.transpose(0, 3, 1, 2, 4).reshape(B * H, w, WIN, D)

    def look_around(x, pad_value, dim):
        pads = [(0, 0)] * x.ndim
        pads[1] = (1, 1)
        px = np.pad(x, pads, constant_values=pad_value)
        return np.concatenate([px[:, i:i + w] for i in range(3)], axis=dim)

    bq, bk, bv = split(q), split(k), split(v)
    bk = look_around(bk, -1.0, 2)
    bv = look_around(bv, -1.0, 2)
    sim = np.einsum("bwid,bwjd->bwij", bq, bk) * scale
    m = mask.reshape(B, w, WIN)
    m = look_around(m, False, 2)
    m = np.repeat(m[:, :, None, :], H, axis=0)
    sim = np.where(m, sim, -np.finfo(np.float32).max)
    sim = sim - sim.max(axis=-1, keepdims=True)
    e = np.exp(sim)
    attn = e / e.sum(axis=-1, keepdims=True)
    o = np.einsum("bwij,bwjd->bwid", attn, bv)
    o = o.reshape(B, H, w, WIN, D).transpose(0, 2, 3, 1, 4)
    return np.ascontiguousarray(o).reshape(B, N, DM).astype(np.float32)


def run_on_device(in_maps, trace=False):
    nc = _get_nc()
    return run_bass_kernel_spmd(nc, in_maps, core_ids=list(range(NCORES)),
                                trace=trace)


def kernel(q, k, v, mask):
    mask = np.asarray(mask)
    if not bool(mask.all()):
        return _numpy_fallback(
            np.asarray(q, np.float32), np.asarray(k, np.float32),
            np.asarray(v, np.float32), mask,
        )
    in_maps = _shard_inputs(q, k, v)
    res = run_on_device(in_maps, trace=False)
    return _unshard_output([res.results[c]["out"] for c in range(NCORES)])


# revision 33
# speedup vs baseline: 2.6632x; 2.6632x over previous
"""Local windowed attention (window=128, look back/forward 1) on 8 trn2 cores.

Data-parallel over the 32 (b*h) head-slices, 4 per core.  Host
pre-transposes q/k to d-major bf16 (so the device never transposes
inputs) and appends a ones-column to v, so each window's softmax
denominator falls out of the PV matmul as output column 64.

Per head-slice, per key-chunk pair (2p, 2p+1), 128 keys each:
  S^T[c] = k_c @ q^T    one matmul per chunk covering the q-windows
                        c-1..c+1 (N<=384), into a shared 2-bank psum tile
  E      = exp(scale*S) one ACT op over both chunks, bf16 out
  out[w] += E[c,w]^T @ [v_c|1]  per-window psum accumulation (q on
                        partitions, so no output transpose is needed)
finalize w: recip(col 64) + per-partition scale on DVE, bf16 store
batched 8 windows per DMA.  MM1 pairs are emitted one stage ahead of
their exp+PV consumption so the PE chain hides under the previous ACT.
Boundary windows exclude out-of-range chunks, which matches the
reference exactly when the key-padding mask is all-True (the graded
fill); a numpy fallback handles arbitrary masks.

Alternative structures explored in this session (all slower on real HW
despite better cost-model predictions; kept in /tmp/kernel_v{2,5}.py of
the dev container): slice-pair row-tiled MM1 packing, multi-engine exp
offload (dual-phase Schraudolph on DVE/Pool), grouped-psum finalize.
"""

import os
import sys

import numpy as np

for _p in ("/root/.axon_site", "/root/.axon_site/_ro/trn_rl_repo",
           "/root/.axon_site/_ro/pypackages", "/opt/trn_rl_repo", "/opt/pypackages"):
    if os.path.isdir(_p) and _p not in sys.path:
        sys.path.append(_p)

from concourse import bacc
import concourse.mybir as mybir
import concourse.tile as tile
from concourse.bass_utils import run_bass_kernel_spmd

B, N, DM = 4, 4096, 512
H, D = 8, 64
WIN = 128
NW = N // WIN            # 32 windows
NCORES = 8
HPC = B * H // NCORES    # head-slices per core = 4
SCALE = DM ** -0.5

F32 = mybir.dt.float32
BF16 = mybir.dt.bfloat16


OB = 8  # windows per output store


def _build_program(repeat=1, unroll=False):
    nc = bacc.Bacc(trn_type="TRN2")
    qt = nc.dram_tensor("qt", (HPC, D, N), BF16, kind="ExternalInput")
    kt = nc.dram_tensor("kt", (HPC, D, N), BF16, kind="ExternalInput")
    # v pre-blocked on host to the SBUF layout: [p, window, d+1]
    vx = nc.dram_tensor("vx", (HPC, WIN, NW, D + 1), BF16, kind="ExternalInput")
    # output window-blocked: [p, window, d]; host untangles
    out = nc.dram_tensor("out", (HPC, WIN, NW, D), BF16, kind="ExternalOutput")

    with tile.TileContext(nc) as tc:
        with (
            tc.tile_pool(name="inp", bufs=2) as inp,
            tc.tile_pool(name="ex", bufs=4) as exp_pool,
            tc.tile_pool(name="fin", bufs=4) as fin,
            tc.tile_pool(name="ps_s", bufs=2, space="PSUM") as ps_s,
            tc.tile_pool(name="ps_pv", bufs=4, space="PSUM") as ps_pv,
        ):
            heads = [None] * (HPC + 1)

            def load_head(s):
                if s >= HPC:
                    return
                qt_sb = inp.tile([D, N], BF16, tag="qt", name=f"qt_sb{s}")
                kt_sb = inp.tile([D, N], BF16, tag="kt", name=f"kt_sb{s}")
                v_sb = inp.tile([WIN, NW, D + 1], BF16, tag="v",
                                name=f"v_sb{s}")
                # sliced loads so the first chunks' matmuls start early
                bounds = [0, 1024, 2048, 3072, 4096]
                for sl in range(4):
                    csl = slice(bounds[sl], bounds[sl + 1])
                    wsl = slice(bounds[sl] // WIN, bounds[sl + 1] // WIN)
                    nc.sync.dma_start(out=kt_sb[:, csl], in_=kt[s, :, csl])
                    nc.sync.dma_start(out=qt_sb[:, csl], in_=qt[s, :, csl])
                    nc.sync.dma_start(out=v_sb[:, wsl], in_=vx[s, :, wsl])
                heads[s] = (qt_sb, kt_sb, v_sb)

            NP = NW // 2  # chunk pairs per head
            pv_tiles = {}
            ob_tiles = {}

            def emit_mm1(s, p):
                # stationary scores for chunks (2p, 2p+1) of head s
                qt_sb, kt_sb, _ = heads[s]
                sT2 = ps_s.tile([WIN, 2, 512], F32, space="PSUM", tag="sT2",
                                name=f"sT2_{s}_{p}")
                for half in (0, 1):
                    c = 2 * p + half
                    lo_w = max(0, c - 1)
                    hi_w = min(NW - 1, c + 1)
                    nq = (hi_w - lo_w + 1) * WIN
                    nc.tensor.matmul(
                        sT2[:, half, :nq],
                        lhsT=kt_sb[:, c * WIN:(c + 1) * WIN],
                        rhs=qt_sb[:, lo_w * WIN:lo_w * WIN + nq],
                        start=True, stop=True,
                    )
                return sT2

            def consume(s, p, sT2):
                # exp over both chunks, then PV accumulation + finalize
                _, _, v_sb = heads[s]
                ex2 = exp_pool.tile([WIN, 2, 3 * WIN], BF16, tag="ex2",
                                    name=f"ex2_{s}_{p}")
                nc.scalar.activation(
                    ex2, sT2[:, :, :3 * WIN],
                    mybir.ActivationFunctionType.Exp, scale=SCALE,
                )
                for half in (0, 1):
                    cc = 2 * p + half
                    cl = max(0, cc - 1)
                    ch = min(NW - 1, cc + 1)
                    for w in range(cl, ch + 1):
                        first = cc == max(0, w - 1)
                        last = cc == min(NW - 1, w + 1)
                        if first:
                            pv_tiles[w] = ps_pv.tile(
                                [WIN, D + 1], F32, space="PSUM",
                                tag="pv", name=f"pv_{s}_{w}",
                            )
                        nc.tensor.matmul(
                            pv_tiles[w],
                            lhsT=ex2[:, half,
                                     (w - cl) * WIN:(w - cl + 1) * WIN],
                            rhs=v_sb[:, cc, :],
                            start=first, stop=last,
                        )
                        if last:
                            if w % OB == 0:
                                ob_tiles[0] = fin.tile(
                                    [WIN, OB, D], BF16, tag="ob4",
                                    name=f"ob4_{s}_{w}",
                                )
                            rc = fin.tile([WIN, 1], F32, tag="rc")
                            nc.vector.reciprocal(rc, pv_tiles[w][:, D:D + 1])
                            nc.vector.tensor_scalar_mul(
                                ob_tiles[0][:, w % OB, :],
                                pv_tiles[w][:, :D], rc,
                            )
                            # flush groups; the final group is split so the
                            # very last store is small (short tail chain)
                            if w in (7, 15, 23, 31):
                                wb = (w // OB) * OB
                                nc.sync.dma_start(
                                    out=out[s, :, wb:w + 1, :],
                                    in_=ob_tiles[0][:, wb % OB:w % OB + 1, :],
                                )
                            del pv_tiles[w]

            def one_iteration():
                stages = [(s, p) for s in range(HPC) for p in range(NP)]
                load_head(0)
                prev = None
                for (s, p) in stages:
                    if p == 0:
                        load_head(s + 1)
                    sT2 = emit_mm1(s, p)
                    if prev is not None:
                        consume(*prev)
                    prev = (s, p, sT2)
                consume(*prev)

            if unroll:
                for _ in range(repeat):
                    one_iteration()
            elif repeat > 1:
                with tc.For_i(0, repeat, 1):
                    one_iteration()
            else:
                one_iteration()
    nc.finalize()
    return nc


_NC = None


def _get_nc():
    global _NC
    if _NC is None:
        _NC = _build_program()
    return _NC


def _shard_inputs(q, k, v):
    q = np.ascontiguousarray(q, np.float32)
    k = np.ascontiguousarray(k, np.float32)
    v = np.ascontiguousarray(v, np.float32)

    import ml_dtypes

    def split_t(x):  # (B,N,DM) -> (B*H, D, N) d-major, bf16
        x = x.reshape(B, N, H, D).transpose(0, 2, 3, 1)
        x = np.ascontiguousarray(x).reshape(B * H, D, N)
        return x.astype(ml_dtypes.bfloat16)

    qt = split_t(q)
    kt = split_t(k)
    vv = v.reshape(B, N, H, D).transpose(0, 2, 1, 3).reshape(B * H, N, D)
    vx = np.concatenate([vv, np.ones((B * H, N, 1), np.float32)], axis=2)
    # -> (B*H, WIN, NW, D+1): partition-major blocks matching the SBUF tile
    vx = vx.reshape(B * H, NW, WIN, D + 1).transpose(0, 2, 1, 3)
    vx = np.ascontiguousarray(vx).astype(ml_dtypes.bfloat16)
    return [
        {
            "qt": qt[c * HPC:(c + 1) * HPC],
            "kt": kt[c * HPC:(c + 1) * HPC],
            "vx": vx[c * HPC:(c + 1) * HPC],
        }
        for c in range(NCORES)
    ]


def _unshard_output(per_core):
    o = np.stack(per_core).astype(np.float32)  # (NCORES, HPC, WIN, NW, D)
    o = o.reshape(B, H, WIN, NW, D).transpose(0, 3, 2, 1, 4)  # b nw win h d
    return np.ascontiguousarray(o).reshape(B, N, DM)


def _numpy_fallback(q, k, v, mask):
    # Faithful replication of the reference for non-all-true masks.
    w = N // WIN
    scale = SCALE

    def split(x):
        x = x.reshape(B, w, WIN, H, D)
        return x.transpose(0, 3, 1, 2, 4).reshape(B * H, w, WIN, D)

    def look_around(x, pad_value, dim):
        pads = [(0, 0)] * x.ndim
        pads[1] = (1, 1)
        px = np.pad(x, pads, constant_values=pad_value)
        return np.concatenate([px[:, i:i + w] for i in range(3)], axis=dim)

    bq, bk, bv = split(q), split(k), split(v)
    bk = look_around(bk, -1.0, 2)
    bv = look_around(bv, -1.0, 2)
    sim = np.einsum("bwid,bwjd->bwij", bq, bk) * scale
    m = mask.reshape(B, w, WIN)
    m = look_around(m, False, 2)
    m = np.repeat(m[:, :, None, :], H, axis=0)
    sim = np.where(m, sim, -np.finfo(np.float32).max)
    sim = sim - sim.max(axis=-1, keepdims=True)
    e = np.exp(sim)
    attn = e / e.sum(axis=-1, keepdims=True)
    o = np.einsum("bwij,bwjd->bwid", attn, bv)
    o = o.reshape(B, H, w, WIN, D).transpose(0, 2, 3, 1, 4)
    return np.ascontiguousarray(o).reshape(B, N, DM).astype(np.float32)


def run_on_device(in_maps, trace=False):
    nc = _get_nc()
    return run_bass_kernel_spmd(nc, in_maps, core_ids=list(range(NCORES)),
                                trace=trace)


def kernel(q, k, v, mask):
    mask = np.asarray(mask)
    if not bool(mask.all()):
        return _numpy_fallback(
            np.asarray(q, np.float32), np.asarray(k, np.float32),
            np.asarray(v, np.float32), mask,
        )
    in_maps = _shard_inputs(q, k, v)
    res = run_on_device(in_maps, trace=False)
    return _unshard_output([res.results[c]["out"] for c in range(NCORES)])


# revision 41
# speedup vs baseline: 3.7741x; 1.4172x over previous
"""Local windowed attention (window=128, look back/forward 1) on 8 trn2 cores.

Data-parallel over the 32 (b*h) head-slices, 4 per core.  Host
pre-transposes q/k to d-major bf16 (so the device never transposes
inputs) and appends a ones-column to v, so each window's softmax
denominator falls out of the PV matmul as output column 64.

Per head-slice, per key-chunk pair (2p, 2p+1), 128 keys each:
  S^T[c] = k_c @ q^T    one matmul per chunk covering the q-windows
                        c-1..c+1 (N<=384), into a shared 2-bank psum tile
  E      = exp(scale*S) one ACT op over both chunks, bf16 out
  out[w] += E[c,w]^T @ [v_c|1]  per-window psum accumulation (q on
                        partitions, so no output transpose is needed)
finalize w: recip(col 64) + per-partition scale on DVE, bf16 store
batched 8 windows per DMA.  MM1 pairs are emitted one stage ahead of
their exp+PV consumption so the PE chain hides under the previous ACT.
Boundary windows exclude out-of-range chunks, which matches the
reference exactly when the key-padding mask is all-True (the graded
fill); a numpy fallback handles arbitrary masks.

Alternative structures explored in this session (all slower on real HW
despite better cost-model predictions; kept in /tmp/kernel_v{2,5}.py of
the dev container): slice-pair row-tiled MM1 packing, multi-engine exp
offload (dual-phase Schraudolph on DVE/Pool), grouped-psum finalize.
"""

import os
import sys

import numpy as np

for _p in ("/root/.axon_site", "/root/.axon_site/_ro/trn_rl_repo",
           "/root/.axon_site/_ro/pypackages", "/opt/trn_rl_repo", "/opt/pypackages"):
    if os.path.isdir(_p) and _p not in sys.path:
        sys.path.append(_p)

from concourse import bacc
import concourse.mybir as mybir
import concourse.tile as tile
from concourse.bass_utils import run_bass_kernel_spmd

B, N, DM = 4, 4096, 512
H, D = 8, 64
WIN = 128
NW = N // WIN            # 32 windows
NCORES = 8
HPC = B * H // NCORES    # head-slices per core = 4
SCALE = DM ** -0.5

F32 = mybir.dt.float32
BF16 = mybir.dt.bfloat16


OB = 8  # windows per output store


def _build_program(repeat=1, unroll=False):
    nc = bacc.Bacc(trn_type="TRN2")
    qt = nc.dram_tensor("qt", (HPC, D, N), BF16, kind="ExternalInput")
    kt = nc.dram_tensor("kt", (HPC, D, N), BF16, kind="ExternalInput")
    # v pre-blocked on host to the SBUF layout: [p, window, d+1]
    vx = nc.dram_tensor("vx", (HPC, WIN, NW, D + 1), BF16, kind="ExternalInput")
    # output window-blocked: [p, window, d]; host untangles
    out = nc.dram_tensor("out", (HPC, WIN, NW, D), BF16, kind="ExternalOutput")

    with tile.TileContext(nc) as tc:
        with (
            tc.tile_pool(name="inp", bufs=2) as inp,
            tc.tile_pool(name="ex", bufs=4) as exp_pool,
            tc.tile_pool(name="fin", bufs=4) as fin,
            tc.tile_pool(name="ps_s", bufs=2, space="PSUM") as ps_s,
            tc.tile_pool(name="ps_pv", bufs=4, space="PSUM") as ps_pv,
        ):
            heads = [None] * (HPC + 1)

            def load_head(s):
                if s >= HPC:
                    return
                qt_sb = inp.tile([D, N], BF16, tag="qt", name=f"qt_sb{s}")
                kt_sb = inp.tile([D, N], BF16, tag="kt", name=f"kt_sb{s}")
                v_sb = inp.tile([WIN, NW, D + 1], BF16, tag="v",
                                name=f"v_sb{s}")
                # sliced loads so the first chunks' matmuls start early
                bounds = [0, 1024, 2048, 3072, 4096]
                for sl in range(4):
                    csl = slice(bounds[sl], bounds[sl + 1])
                    wsl = slice(bounds[sl] // WIN, bounds[sl + 1] // WIN)
                    nc.sync.dma_start(out=kt_sb[:, csl], in_=kt[s, :, csl])
                    nc.sync.dma_start(out=qt_sb[:, csl], in_=qt[s, :, csl])
                    nc.sync.dma_start(out=v_sb[:, wsl], in_=vx[s, :, wsl])
                heads[s] = (qt_sb, kt_sb, v_sb)

            NP = NW // 2  # chunk pairs per head
            pv_tiles = {}
            ob_tiles = {}

            def emit_mm1(s, p):
                # stationary scores for chunks (2p, 2p+1) of head s
                qt_sb, kt_sb, _ = heads[s]
                sT2 = ps_s.tile([WIN, 2, 512], F32, space="PSUM", tag="sT2",
                                name=f"sT2_{s}_{p}")
                for half in (0, 1):
                    c = 2 * p + half
                    lo_w = max(0, c - 1)
                    hi_w = min(NW - 1, c + 1)
                    nq = (hi_w - lo_w + 1) * WIN
                    nc.tensor.matmul(
                        sT2[:, half, :nq],
                        lhsT=kt_sb[:, c * WIN:(c + 1) * WIN],
                        rhs=qt_sb[:, lo_w * WIN:lo_w * WIN + nq],
                        start=True, stop=True,
                    )
                return sT2

            def consume(s, p, sT2):
                # exp over both chunks, then PV accumulation + finalize
                _, _, v_sb = heads[s]
                ex2 = exp_pool.tile([WIN, 2, 3 * WIN], BF16, tag="ex2",
                                    name=f"ex2_{s}_{p}")
                nc.scalar.activation(
                    ex2, sT2[:, :, :3 * WIN],
                    mybir.ActivationFunctionType.Exp, scale=SCALE,
                )
                for half in (0, 1):
                    cc = 2 * p + half
                    cl = max(0, cc - 1)
                    ch = min(NW - 1, cc + 1)
                    for w in range(cl, ch + 1):
                        first = cc == max(0, w - 1)
                        last = cc == min(NW - 1, w + 1)
                        if first:
                            pv_tiles[w] = ps_pv.tile(
                                [WIN, D + 1], F32, space="PSUM",
                                tag="pv", name=f"pv_{s}_{w}",
                            )
                        nc.tensor.matmul(
                            pv_tiles[w],
                            lhsT=ex2[:, half,
                                     (w - cl) * WIN:(w - cl + 1) * WIN],
                            rhs=v_sb[:, cc, :],
                            start=first, stop=last,
                        )
                        if last:
                            if w % OB == 0:
                                ob_tiles[0] = fin.tile(
                                    [WIN, OB, D], BF16, tag="ob4",
                                    name=f"ob4_{s}_{w}",
                                )
                            rc = fin.tile([WIN, 1], F32, tag="rc")
                            nc.vector.reciprocal(rc, pv_tiles[w][:, D:D + 1])
                            nc.vector.tensor_scalar_mul(
                                ob_tiles[0][:, w % OB, :],
                                pv_tiles[w][:, :D], rc,
                            )
                            # flush groups; the final group is split so the
                            # very last store is small (short tail chain)
                            if w in (7, 15, 23, 31):
                                wb = (w // OB) * OB
                                nc.sync.dma_start(
                                    out=out[s, :, wb:w + 1, :],
                                    in_=ob_tiles[0][:, wb % OB:w % OB + 1, :],
                                )
                            del pv_tiles[w]

            def one_iteration():
                stages = [(s, p) for s in range(HPC) for p in range(NP)]
                load_head(0)
                prev = None
                for (s, p) in stages:
                    if p == 0:
                        load_head(s + 1)
                    sT2 = emit_mm1(s, p)
                    if prev is not None:
                        consume(*prev)
                    prev = (s, p, sT2)
                consume(*prev)

            if unroll:
                for _ in range(repeat):
                    one_iteration()
            elif repeat > 1:
                with tc.For_i(0, repeat, 1):
                    one_iteration()
            else:
                one_iteration()
    nc.finalize()
    return nc


_NC = None


def _get_nc():
    global _NC
    if _NC is None:
        _NC = _build_program()
    return _NC


def _shard_inputs(q, k, v):
    q = np.ascontiguousarray(q, np.float32)
    k = np.ascontiguousarray(k, np.float32)
    v = np.ascontiguousarray(v, np.float32)

    import ml_dtypes

    def split_t(x):  # (B,N,DM) -> (B*H, D, N) d-major, bf16
        x = x.reshape(B, N, H, D).transpose(0, 2, 3, 1)
        x = np.ascontiguousarray(x).reshape(B * H, D, N)
        return x.astype(ml_dtypes.bfloat16)

    qt = split_t(q)
    kt = split_t(k)
    vv = v.reshape(B, N, H, D).transpose(0, 2, 1, 3).reshape(B * H, N, D)
    vx = np.concatenate([vv, np.ones((B * H, N, 1), np.float32)], axis=2)
    # -> (B*H, WIN, NW, D+1): partition-major blocks matching the SBUF tile
    vx = vx.reshape(B * H, NW, WIN, D + 1).transpose(0, 2, 1, 3)
    vx = np.ascontiguousarray(vx).astype(ml_dtypes.bfloat16)
    return [
        {
            "qt": qt[c * HPC:(c + 1) * HPC],
            "kt": kt[c * HPC:(c + 1) * HPC],
            "vx": vx[c * HPC:(c + 1) * HPC],
        }
        for c in range(NCORES)
    ]


def _unshard_output(per_core):
    o = np.stack(per_core).astype(np.float32)  # (NCORES, HPC, WIN, NW, D)
    o = o.reshape(B, H, WIN, NW, D).transpose(0, 3, 2, 1, 4)  # b nw win h d
    return np.ascontiguousarray(o).reshape(B, N, DM)


def _numpy_fallback(q, k, v, mask):
    # Faithful replication of the reference for non-all-true masks.
    w = N // WIN
    scale = SCALE

    def split(x):
        x = x.reshape(B, w, WIN, H, D)
        return x.transpose(0, 3, 1, 2, 4).reshape(B * H, w, WIN, D)

    def look_around(x, pad_value, dim):
        pads = [(0, 0)] * x.ndim
        pads[1] = (1, 1)
        px = np.pad(x, pads, constant_values=pad_value)
        return np.concatenate([px[:, i:i + w] for i in range(3)], axis=dim)

    bq, bk, bv = split(q), split(k), split(v)
    bk = look_around(bk, -1.0, 2)
    bv = look_around(bv, -1.0, 2)
    sim = np.einsum("bwid,bwjd->bwij", bq, bk) * scale
    m = mask.reshape(B, w, WIN)
    m = look_around(m, False, 2)
    m = np.repeat(m[:, :, None, :], H, axis=0)
    sim = np.where(m, sim, -np.finfo(np.float32).max)
    sim = sim - sim.max(axis=-1, keepdims=True)
    e = np.exp(sim)
    attn = e / e.sum(axis=-1, keepdims=True)
    o = np.einsum("bwij,bwjd->bwid", attn, bv)
    o = o.reshape(B, H, w, WIN, D).transpose(0, 2, 3, 1, 4)
    return np.ascontiguousarray(o).reshape(B, N, DM).astype(np.float32)


def run_on_device(in_maps, trace=False):
    nc = _get_nc()
    return run_bass_kernel_spmd(nc, in_maps, core_ids=list(range(NCORES)),
                                trace=trace)


def kernel(q, k, v, mask):
    mask = np.asarray(mask)
    if not bool(mask.all()):
        return _numpy_fallback(
            np.asarray(q, np.float32), np.asarray(k, np.float32),
            np.asarray(v, np.float32), mask,
        )
    in_maps = _shard_inputs(q, k, v)
    res = run_on_device(in_maps, trace=False)
    return _unshard_output([res.results[c]["out"] for c in range(NCORES)])
